# revision 1
# baseline (speedup 1.0000x reference)
"""GNN message-passing NodeBlock kernel for 8 Trainium2 NeuronCores.

Problem:
    agg_a = segment_sum(edata_a, conn_a[1], 100000)   # [N, 64]
    agg_b = segment_sum(edata_b, conn_b[1], 100000)   # [N, 64]
    out   = concat([agg_a, agg_b, vdata], 1) @ W + b  # [N, 128]

Sharding: edges sharded BY RECEIVER - nodes are split into 64-node
windows, and windows are snake-assigned to the 8 cores in descending
tile-count order (the host reassembles the output), so each core
computes its windows' aggregation completely locally (no collective)
and the per-step SPMD padding nearly vanishes. Each 128-edge tile is
scattered into its window via a one-hot selection matrix (is_equal of
rel vs iota) and a PE matmul accumulated in PSUM; the dense updater
follows as 512-col bf16 matmuls over the scatter results.

Precision (gate is 2e-2): edge features travel bf16 (fp8 measured
2.7e-2 - fails), vdata fp8, W/x0 bf16, output stored bf16 and widened
on host. Measured end-to-end rel err 9.6e-3.

Data movement: edge tiles for a whole block arrive as ONE ~1.2 MB DMA
on the sync HWDGE queue (kept free of everything else); consts, vT,
rel and output stores ride the scalar queue. The sel build needs sel
in (t,w) layout (strided matmul rhs costs 2x on PE - measured), but a
(t,w) is_equal against per-tile rel has a stride-0-inner broadcast
operand which drops DVE to its 1x uop. Trick: the host stores rel
DUPLICATED x2 (rel2[p,2t+d] = rel[p,t]); the AP [p, t(2), w32(0),
d(1, size 2)] then has a dense 64-bit innermost run, which keeps the
2x uop (HW-measured 2284ns vs 4418ns at FD=4096). iota rides as a
[128,64] tile broadcast over t.

Schedule: blocks are processed in descending size so the smallest
blocks drain last. Padding slots carry rel=-1 and zero data.
"""
import numpy as np
import ml_dtypes

import concourse.bass as bass
import concourse.tile as tile
from concourse import mybir
from concourse.bass_utils import run_bass_kernel_spmd
from concourse.vector_clock import ScopedClock

BF16 = ml_dtypes.bfloat16

N_NODES = 100000
N_EDGES = 800000
D_EDGE = 64
D_NODE = 128
D_OUT = 128
N_CORES = 8
WIN = 64                   # nodes per window
WPC = 196                  # windows per core
NPC = WIN * WPC            # nodes per core (12544)
NTOT = NPC * N_CORES       # padded node space (100352)
BLK_STEPS = 8              # max windows per phase-2 block (8*64 = 512 cols)
# 24 full blocks + one 4-step tail block (sums to WPC=196); a deeper
# taper was measured slower (per-block overheads beat the drain savings)
BLOCK_PLAN = [8] * 24 + [4]
N_BLKS = len(BLOCK_PLAN)
OUT_PLAN = (13, 8, 4)      # blocks per outT store (small final store)

# ---------------------------------------------------------------------------
# compat patches for this container's walrus build
# ---------------------------------------------------------------------------

_MAX_WAITS = 1


def _patched_drain_and_barrier(self, tick_clock, wait_clock):
    nc = self.nc
    probe = nc.sync.nop(nofuse=True, hint="tile_drain_wait0")
    wait_clock.add_sem_waits(
        probe.ins, ScopedClock({None: tick_clock.global_clock})
    )
    si = probe.ins.sync_info
    waits = list(si.on_wait) if si is not None and si.on_wait else []
    if len(waits) > _MAX_WAITS:
        si.on_wait = waits[:_MAX_WAITS]
        for k in range(_MAX_WAITS, len(waits), _MAX_WAITS):
            n = nc.sync.nop(nofuse=True, hint=f"tile_drain_wait{k}")
            n.ins.sync_info = mybir.SyncInfo(
                on_wait=waits[k : k + _MAX_WAITS], on_update=[]
            )
    drain_inst = nc.sync.drain()
    wait_clock.add_sem_waits(
        drain_inst.ins, ScopedClock({None: tick_clock.global_clock})
    )
    dsi = drain_inst.ins.sync_info
    if dsi is not None and dsi.on_wait and len(dsi.on_wait) > _MAX_WAITS:
        dsi.on_wait = []
    nc.all_engine_barrier()
    assert self.sems is not None
    popped = nc._tile_sem_poison_stack.pop()
    assert popped is self._sem_poison
    nc.clear_and_free_semaphores(list(self.sems.allocated().values()))
    nc.all_engine_barrier()


def _split_multi_waits(nc):
    """This walrus build accepts one sync-wait per TPB instruction; move
    extra waits onto preceding same-engine NOPs."""
    for fn in nc.m.functions:
        for blk in fn.blocks:
            out = []
            changed = False
            for inst in blk.instructions:
                si = inst.sync_info
                if si is not None and si.on_wait and len(si.on_wait) > 1:
                    waits = list(si.on_wait)
                    for j, w in enumerate(waits[:-1]):
                        nop = mybir.InstNoOp(
                            name=f"{inst.name}_xw{j}", ins=[], outs=[]
                        )
                        nop.engine = inst.engine
                        nop.sync_info = mybir.SyncInfo(
                            on_wait=[w], on_update=[]
                        )
                        out.append(nop)
                    si.on_wait = [waits[-1]]
                    changed = True
                out.append(inst)
            if changed:
                blk.instructions = out


def _install_ntff_hook_shim():
    import sys
    import types

    if "antenv.axon_hooks" in sys.modules:
        return
    mod = types.ModuleType("antenv.axon_hooks")
    _hook = [None]
    mod.set_axon_ntff_profile_hook = lambda h: _hook.__setitem__(0, h)
    mod.get_axon_ntff_profile_hook = lambda: _hook[0]
    sys.modules["antenv.axon_hooks"] = mod
    try:
        import antenv

        antenv.axon_hooks = mod
    except ImportError:
        pass
    try:
        from trn_agent_boot.trn_boot import _ntff_profile_via_ctypes

        mod.set_axon_ntff_profile_hook(
            _ntff_profile_via_ctypes("/opt/axon/libaxon_pjrt.so")
        )
    except Exception:
        pass


tile.TileContext._drain_and_barrier = _patched_drain_and_barrier
_install_ntff_hook_shim()

# ---------------------------------------------------------------------------
# host-side sharding / packing
# ---------------------------------------------------------------------------


def _preprocess(vdata, edata_a, edata_b, conn_a, conn_b, W_mat, b_vec):
    recv_a = np.asarray(conn_a[1]).astype(np.int64)
    recv_b = np.asarray(conn_b[1]).astype(np.int64)

    n_win_tot = WPC * N_CORES

    def bin_type(recv):
        gwin = recv >> 6  # global 64-node window id
        order = np.argsort(gwin, kind="stable")
        counts = np.bincount(gwin, minlength=n_win_tot)
        starts = np.zeros(n_win_tot + 1, dtype=np.int64)
        np.cumsum(counts, out=starts[1:])
        return order, counts, starts

    ids_a, cnt_a, st_a = bin_type(recv_a)
    ids_b, cnt_b, st_b = bin_type(recv_b)

    ta_g = np.ceil(cnt_a / 128).astype(np.int64)  # [n_win_tot]
    tb_g = np.ceil(cnt_b / 128).astype(np.int64)
    # windows are free to permute (host reassembles output): snake-assign
    # them to cores in descending tile-count order, which both balances
    # per-core totals and aligns the per-step order statistics so the
    # max-over-cores SPMD padding nearly vanishes
    order = np.argsort(-(ta_g * 1000 + tb_g), kind="stable")
    gwins = np.empty((N_CORES, WPC), dtype=np.int64)
    for i, w in enumerate(order):
        r, k = divmod(i, N_CORES)
        c = k if r % 2 == 0 else N_CORES - 1 - k
        gwins[c, r] = w
    # per-core step order: descending by the same key (r is already that)
    tiles_a = ta_g[gwins]  # [N_CORES, WPC]
    tiles_b = tb_g[gwins]
    na_step = np.maximum(tiles_a.max(axis=0), 1)  # [WPC]
    nb_step = np.maximum(tiles_b.max(axis=0), 1)

    # per-step slot offsets in the packed (a+b interleaved per block) layout:
    # block j holds [a tiles of steps i0..i0+steps) then [b tiles ...]
    step_off_a = np.zeros(WPC, np.int64)
    step_off_b = np.zeros(WPC, np.int64)
    blk_base = 0
    i0 = 0
    for j in range(N_BLKS):
        steps = BLOCK_PLAN[j]
        na_blk = int(na_step[i0 : i0 + steps].sum())
        o = blk_base
        for i in range(i0, i0 + steps):
            step_off_a[i] = o
            o += na_step[i]
        o = blk_base + na_blk
        for i in range(i0, i0 + steps):
            step_off_b[i] = o
            o += nb_step[i]
        blk_base = o
        i0 += steps
    T_tot = int(blk_base)

    ea16 = np.asarray(edata_a).astype(BF16)
    eb16 = np.asarray(edata_b).astype(BF16)

    vdata = np.asarray(vdata)
    vpad = np.zeros((NTOT, D_NODE), dtype=np.float32)
    vpad[:N_NODES] = vdata

    blk_tot = []
    i0 = 0
    for j in range(N_BLKS):
        steps = BLOCK_PLAN[j]
        blk_tot.append(
            int(na_step[i0 : i0 + steps].sum() + nb_step[i0 : i0 + steps].sum())
        )
        i0 += steps
    max_blk = max(blk_tot)

    iota64 = np.ascontiguousarray(
        np.broadcast_to(np.arange(WIN, dtype=np.float32), (128, WIN))
    ).astype(BF16)
    Wf = np.ascontiguousarray(np.asarray(W_mat), dtype=np.float32).astype(BF16)
    bf = np.asarray(b_vec).astype(np.float32).reshape(D_OUT, 1)

    in_maps = []
    for c in range(N_CORES):
        slot_eid = np.full(T_tot * 128, -1, dtype=np.int64)
        slot_rel = np.full(T_tot * 128, -1.0, dtype=np.float32)
        slot_is_a = np.zeros(T_tot * 128, dtype=bool)
        for i in range(WPC):
            g = gwins[c][i]
            for ids, starts, cnts, soff, recv, is_a in (
                (ids_a, st_a, cnt_a, step_off_a, recv_a, True),
                (ids_b, st_b, cnt_b, step_off_b, recv_b, False),
            ):
                cnt = cnts[g]
                if cnt == 0:
                    continue
                eids = ids[starts[g] : starts[g] + cnt]
                s0 = soff[i] * 128
                slot_eid[s0 : s0 + cnt] = eids
                slot_is_a[s0 : s0 + cnt] = is_a
                slot_rel[s0 : s0 + cnt] = (recv[eids] & (WIN - 1)).astype(
                    np.float32
                )
        idx = np.maximum(slot_eid, 0)
        gath = np.where(slot_is_a[:, None], ea16[idx], eb16[idx])
        gath[slot_eid < 0] = 0
        eh = np.ascontiguousarray(
            gath.reshape(T_tot, 128, 64).transpose(1, 0, 2)
        )  # [slot, tile, feat] bf16
        relT = slot_rel.reshape(T_tot, 128).T.astype(BF16)
        rel2 = np.ascontiguousarray(np.repeat(relT, 2, axis=1))  # [128, 2T]
        nodes = (
            gwins[c][:, None] * WIN + np.arange(WIN)[None, :]
        ).reshape(-1)
        vT = np.ascontiguousarray(
            vpad[nodes].T.astype(ml_dtypes.float8_e4m3)
        )  # [128, NPC] fp8
        in_maps.append(
            {"eh": eh, "rel": rel2, "vT": vT, "Wd": Wf, "bd": bf,
             "iota": iota64}
        )

    sched = (tuple(int(x) for x in na_step), tuple(int(x) for x in nb_step))
    return in_maps, sched, gwins


# ---------------------------------------------------------------------------
# device kernel
# ---------------------------------------------------------------------------

_NC_CACHE = {}


def _build(sched):
    na_step, nb_step = sched
    f32 = mybir.dt.float32
    bf16 = mybir.dt.bfloat16

    # packed per-block layout: [a tiles | b tiles] per block
    blk_na = []
    blk_nb = []
    blk_i0 = []
    i0 = 0
    for j in range(N_BLKS):
        steps = BLOCK_PLAN[j]
        blk_i0.append(i0)
        blk_na.append(sum(na_step[i0 : i0 + steps]))
        blk_nb.append(sum(nb_step[i0 : i0 + steps]))
        i0 += steps
    blk_tot = [a + b for a, b in zip(blk_na, blk_nb)]
    max_blk = max(blk_tot)
    T_tot = sum(blk_tot)

    nc = bass.Bass(trn_type="TRN2")
    eh_d = nc.dram_tensor("eh", [128, T_tot, 64], bf16, kind="ExternalInput")
    rel_d = nc.dram_tensor("rel", [128, 2 * T_tot], bf16, kind="ExternalInput")
    fp8 = mybir.dt.float8e4
    vT_d = nc.dram_tensor("vT", [128, NPC], fp8, kind="ExternalInput")
    W_d = nc.dram_tensor("Wd", [2 * D_NODE, D_OUT], bf16, kind="ExternalInput")
    b_d = nc.dram_tensor("bd", [D_OUT, 1], f32, kind="ExternalInput")
    iota_d = nc.dram_tensor("iota", [128, WIN], bf16, kind="ExternalInput")
    outT_d = nc.dram_tensor("outT", [128, NPC], bf16, kind="ExternalOutput")

    with tile.TileContext(nc) as tc:
        with (
            tc.tile_pool(name="consts", bufs=1) as cb,
            tc.tile_pool(name="x0", bufs=3) as x0p,
            tc.tile_pool(name="edges", bufs=5) as ep,
            tc.tile_pool(name="sel", bufs=4) as sp,
            tc.tile_pool(name="out", bufs=2) as op,
            tc.tile_pool(name="psum1", bufs=4, space="PSUM") as pp1,
            tc.tile_pool(name="psum2", bufs=2, space="PSUM") as pp2,
        ):
            iota_sb = cb.tile([128, WIN], bf16)
            nc.scalar.dma_start(iota_sb[:], iota_d[:, :])
            w0_sb = cb.tile([128, D_OUT], bf16, tag="w0")
            nc.scalar.dma_start(w0_sb[:], W_d[0:128, :])
            w1_sb = cb.tile([128, D_OUT], bf16, tag="w1")
            nc.scalar.dma_start(w1_sb[:], W_d[128:256, :])
            b_sb = cb.tile([D_OUT, 1], f32, tag="b")
            nc.scalar.dma_start(b_sb[:], b_d[:, :])
            rel_sb = cb.tile([128, 2 * T_tot], bf16, tag="rel")
            rel_head = 2 * sum(blk_tot[:6])
            nc.scalar.dma_start(rel_sb[:, :rel_head], rel_d[:, :rel_head])
            vt_sb = cb.tile([128, NPC], fp8, tag="vt")

            off = 0
            ot = None
            chunk_col0 = 0
            ot_cols = 0
            chunk_starts = set()
            s = 0
            for n in OUT_PLAN:
                chunk_starts.add(s)
                s += n
            for j in range(N_BLKS):
                i0 = blk_i0[j]
                steps = BLOCK_PLAN[j]
                cols_blk = steps * WIN
                n_blk = blk_tot[j]
                na_b = blk_na[j]

                # one coalesced edge DMA per block (~1.2 MB)
                et = ep.tile([128, max_blk * 64], bf16, tag="et")
                nc.sync.dma_start(
                    et[:, : n_blk * 64], eh_d[:, off : off + n_blk, :]
                )
                if j == 1:
                    nc.scalar.dma_start(
                        rel_sb[:, rel_head:], rel_d[:, rel_head:]
                    )
                # vT arrives in 5 chunks spread over the early blocks
                if j < 10 and j % 2 == 0:
                    k = j // 2
                    vc0 = k * (NPC // 5)
                    vc1 = NPC if k == 4 else (k + 1) * (NPC // 5)
                    nc.scalar.dma_start(vt_sb[:, vc0:vc1], vT_d[:, vc0:vc1])

                selb = sp.tile([128, max_blk * WIN], bf16, tag="selb")
                # rel2 dup-x2 keeps a dense 64-bit innermost run -> 2x uop
                in0 = rel_sb[:, 2 * off : 2 * (off + n_blk)].rearrange(
                    "p (n one d) -> p n one d", one=1, d=2
                ).broadcast_to([128, n_blk, WIN // 2, 2])
                in1 = iota_sb[:].rearrange(
                    "p (w d) -> p w d", d=2
                ).rearrange(
                    "p (one w) d -> p one w d", one=1
                ).broadcast_to([128, n_blk, WIN // 2, 2])
                nc.vector.tensor_tensor(
                    out=selb[:, : n_blk * WIN].rearrange(
                        "p (n w d) -> p n w d", w=WIN // 2, d=2
                    ),
                    in0=in0, in1=in1, op=mybir.AluOpType.is_equal,
                )

                x0 = x0p.tile([128, BLK_STEPS * WIN], bf16, tag="x0")
                ps = pp1.tile([128, BLK_STEPS * WIN], f32, tag="p1")
                t = 0
                for half, n_stp in ((0, na_step), (1, nb_step)):
                    r0 = half * 64
                    tt = 0
                    n_half = blk_na[j] if half == 0 else blk_nb[j]
                    for stp in range(steps):
                        for k in range(n_stp[i0 + stp]):
                            nc.tensor.matmul(
                                out=ps[
                                    r0 : r0 + 64,
                                    stp * WIN : (stp + 1) * WIN,
                                ],
                                lhsT=et[:, t * 64 : (t + 1) * 64],
                                rhs=selb[:, t * WIN : (t + 1) * WIN],
                                start=(tt == 0),
                                stop=(tt == n_half - 1),
                            )
                            t += 1
                            tt += 1
                nc.scalar.copy(x0[:, :cols_blk], ps[:, :cols_blk])
                off += n_blk

                po = pp2.tile([128, BLK_STEPS * WIN], f32, tag="p2")
                nc.tensor.matmul(
                    out=po[:, :cols_blk], lhsT=w0_sb[:], rhs=x0[:, :cols_blk],
                    start=True, stop=False,
                )
                nc.tensor.matmul(
                    out=po[:, :cols_blk],
                    lhsT=w1_sb[:],
                    rhs=vt_sb[:, i0 * WIN : i0 * WIN + cols_blk],
                    start=False, stop=True,
                )
                # outT staged per OUT_PLAN chunk, then one store each
                if j in chunk_starts:
                    ot = op.tile(
                        [128, max(OUT_PLAN) * BLK_STEPS * WIN], bf16, tag="ot"
                    )
                    chunk_col0 = i0 * WIN
                    ot_cols = 0
                nc.scalar.activation(
                    out=ot[:, ot_cols : ot_cols + cols_blk],
                    in_=po[:, :cols_blk],
                    func=mybir.ActivationFunctionType.Identity,
                    bias=b_sb[:, 0:1],
                    scale=1.0,
                )
                ot_cols += cols_blk
                if j + 1 in chunk_starts or j == N_BLKS - 1:
                    nc.scalar.dma_start(
                        outT_d[:, chunk_col0 : chunk_col0 + ot_cols],
                        ot[:, :ot_cols],
                    )
    _split_multi_waits(nc)
    return nc


# ---------------------------------------------------------------------------
# public entry point
# ---------------------------------------------------------------------------


def kernel(vdata, edata_a, edata_b, conn_a, conn_b, W, b, _trace=False):
    in_maps, sched, gwins = _preprocess(
        vdata, edata_a, edata_b, conn_a, conn_b, W, b
    )
    nc = _NC_CACHE.get(sched)
    if nc is None:
        nc = _build(sched)
        _NC_CACHE[sched] = nc
    kwargs = {}
    if _trace:
        kwargs = dict(trace=True, trace_cores=[0])
    res = run_bass_kernel_spmd(
        nc, in_maps, core_ids=list(range(N_CORES)), **kwargs
    )

    out_full = np.empty((NTOT, D_OUT), dtype=np.float32)
    for c in range(N_CORES):
        outT = res.results[c]["outT"].astype(np.float32)  # [128, NPC] bf16
        blocks = outT.reshape(D_OUT, WPC, WIN).transpose(1, 2, 0)  # [i, w, d]
        # scatter step blocks back to natural (global window) order
        dst = gwins[c][:, None] * WIN + np.arange(WIN)[None, :]
        out_full[dst.reshape(-1)] = blocks.reshape(NPC, D_OUT)
    out = out_full[:N_NODES]
    if _trace:
        return out, res
    return out



# revision 6
# speedup vs baseline: 1.2301x; 1.2301x over previous
"""GNN message-passing NodeBlock kernel for 8 Trainium2 NeuronCores.

Problem:
    agg_a = segment_sum(edata_a, conn_a[1], 100000)   # [N, 64]
    agg_b = segment_sum(edata_b, conn_b[1], 100000)   # [N, 64]
    out   = concat([agg_a, agg_b, vdata], 1) @ W + b  # [N, 128]

Sharding: edges sharded BY RECEIVER - nodes are split into 64-node
windows, and windows are snake-assigned to the 8 cores in descending
tile-count order (the host reassembles the output), so each core
computes its windows' aggregation completely locally (no collective)
and the per-step SPMD padding nearly vanishes. Each 128-edge tile is
scattered into its window via a one-hot selection matrix (is_equal of
rel vs iota) and a PE matmul accumulated in PSUM; the dense updater
follows as 512-col bf16 matmuls over the scatter results.

Precision (gate is 2e-2): edge features travel bf16 (fp8 measured
2.7e-2 - fails), vdata fp8, W/x0 bf16, output stored bf16 and widened
on host. Measured end-to-end rel err 9.6e-3.

Data movement: edge tiles for a whole block arrive as ONE ~1.2 MB DMA
on the sync HWDGE queue (kept free of everything else); consts, vT,
rel and output stores ride the scalar queue. The sel build needs sel
in (t,w) layout (strided matmul rhs costs 2x on PE - measured), but a
(t,w) is_equal against per-tile rel has a stride-0-inner broadcast
operand which drops DVE to its 1x uop. Trick: the host stores rel
DUPLICATED x2 (rel2[p,2t+d] = rel[p,t]); the AP [p, t(2), w32(0),
d(1, size 2)] then has a dense 64-bit innermost run, which keeps the
2x uop (HW-measured 2284ns vs 4418ns at FD=4096). iota rides as a
[128,64] tile broadcast over t.

Schedule: blocks are processed in descending size so the smallest
blocks drain last. Padding slots carry rel=-1 and zero data.
"""
import numpy as np
import ml_dtypes

import concourse.bass as bass
import concourse.tile as tile
from concourse import mybir
from concourse.bass_utils import run_bass_kernel_spmd
from concourse.vector_clock import ScopedClock

BF16 = ml_dtypes.bfloat16
E3M4 = ml_dtypes.float8_e3m4

N_NODES = 100000
N_EDGES = 800000
D_EDGE = 64
D_NODE = 128
D_OUT = 128
N_CORES = 8
WIN = 64                   # nodes per window
WPC = 196                  # windows per core
NPC = WIN * WPC            # nodes per core (12544)
NTOT = NPC * N_CORES       # padded node space (100352)
BLK_STEPS = 8              # max windows per phase-2 block (8*64 = 512 cols)
# 24 full blocks + one 4-step tail block (sums to WPC=196); a deeper
# taper was measured slower (per-block overheads beat the drain savings)
BLOCK_PLAN = [8] * 24 + [4]
N_BLKS = len(BLOCK_PLAN)
OUT_PLAN = (13, 8, 4)      # blocks per outT store (small final store)

# ---------------------------------------------------------------------------
# compat patches for this container's walrus build
# ---------------------------------------------------------------------------

_MAX_WAITS = 1


def _patched_drain_and_barrier(self, tick_clock, wait_clock):
    nc = self.nc
    probe = nc.sync.nop(nofuse=True, hint="tile_drain_wait0")
    wait_clock.add_sem_waits(
        probe.ins, ScopedClock({None: tick_clock.global_clock})
    )
    si = probe.ins.sync_info
    waits = list(si.on_wait) if si is not None and si.on_wait else []
    if len(waits) > _MAX_WAITS:
        si.on_wait = waits[:_MAX_WAITS]
        for k in range(_MAX_WAITS, len(waits), _MAX_WAITS):
            n = nc.sync.nop(nofuse=True, hint=f"tile_drain_wait{k}")
            n.ins.sync_info = mybir.SyncInfo(
                on_wait=waits[k : k + _MAX_WAITS], on_update=[]
            )
    drain_inst = nc.sync.drain()
    wait_clock.add_sem_waits(
        drain_inst.ins, ScopedClock({None: tick_clock.global_clock})
    )
    dsi = drain_inst.ins.sync_info
    if dsi is not None and dsi.on_wait and len(dsi.on_wait) > _MAX_WAITS:
        dsi.on_wait = []
    nc.all_engine_barrier()
    assert self.sems is not None
    popped = nc._tile_sem_poison_stack.pop()
    assert popped is self._sem_poison
    nc.clear_and_free_semaphores(list(self.sems.allocated().values()))
    nc.all_engine_barrier()


def _split_multi_waits(nc):
    """This walrus build accepts one sync-wait per TPB instruction; move
    extra waits onto preceding same-engine NOPs."""
    for fn in nc.m.functions:
        for blk in fn.blocks:
            out = []
            changed = False
            for inst in blk.instructions:
                si = inst.sync_info
                if si is not None and si.on_wait and len(si.on_wait) > 1:
                    waits = list(si.on_wait)
                    for j, w in enumerate(waits[:-1]):
                        nop = mybir.InstNoOp(
                            name=f"{inst.name}_xw{j}", ins=[], outs=[]
                        )
                        nop.engine = inst.engine
                        nop.sync_info = mybir.SyncInfo(
                            on_wait=[w], on_update=[]
                        )
                        out.append(nop)
                    si.on_wait = [waits[-1]]
                    changed = True
                out.append(inst)
            if changed:
                blk.instructions = out


def _install_ntff_hook_shim():
    import sys
    import types

    if "antenv.axon_hooks" in sys.modules:
        return
    mod = types.ModuleType("antenv.axon_hooks")
    _hook = [None]
    mod.set_axon_ntff_profile_hook = lambda h: _hook.__setitem__(0, h)
    mod.get_axon_ntff_profile_hook = lambda: _hook[0]
    sys.modules["antenv.axon_hooks"] = mod
    try:
        import antenv

        antenv.axon_hooks = mod
    except ImportError:
        pass
    try:
        from trn_agent_boot.trn_boot import _ntff_profile_via_ctypes

        mod.set_axon_ntff_profile_hook(
            _ntff_profile_via_ctypes("/opt/axon/libaxon_pjrt.so")
        )
    except Exception:
        pass


tile.TileContext._drain_and_barrier = _patched_drain_and_barrier
_install_ntff_hook_shim()

# ---------------------------------------------------------------------------
# host-side sharding / packing
# ---------------------------------------------------------------------------


def _preprocess(vdata, edata_a, edata_b, conn_a, conn_b, W_mat, b_vec):
    recv_a = np.asarray(conn_a[1]).astype(np.int64)
    recv_b = np.asarray(conn_b[1]).astype(np.int64)

    n_win_tot = WPC * N_CORES

    def bin_type(recv):
        gwin = recv >> 6  # global 64-node window id
        order = np.argsort(gwin, kind="stable")
        counts = np.bincount(gwin, minlength=n_win_tot)
        starts = np.zeros(n_win_tot + 1, dtype=np.int64)
        np.cumsum(counts, out=starts[1:])
        return order, counts, starts

    ids_a, cnt_a, st_a = bin_type(recv_a)
    ids_b, cnt_b, st_b = bin_type(recv_b)

    ta_g = np.ceil(cnt_a / 128).astype(np.int64)  # [n_win_tot]
    tb_g = np.ceil(cnt_b / 128).astype(np.int64)
    # windows are free to permute (host reassembles output): snake-assign
    # them to cores in descending tile-count order, which both balances
    # per-core totals and aligns the per-step order statistics so the
    # max-over-cores SPMD padding nearly vanishes
    order = np.argsort(-(ta_g * 1000 + tb_g), kind="stable")
    gwins = np.empty((N_CORES, WPC), dtype=np.int64)
    for i, w in enumerate(order):
        r, k = divmod(i, N_CORES)
        c = k if r % 2 == 0 else N_CORES - 1 - k
        gwins[c, r] = w
    # per-core step order: descending by the same key (r is already that)
    tiles_a = ta_g[gwins]  # [N_CORES, WPC]
    tiles_b = tb_g[gwins]
    na_step = np.maximum(tiles_a.max(axis=0), 1)  # [WPC]
    nb_step = np.maximum(tiles_b.max(axis=0), 1)

    # per-step slot offsets in the packed (a+b interleaved per block) layout:
    # block j holds [a tiles of steps i0..i0+steps) then [b tiles ...]
    step_off_a = np.zeros(WPC, np.int64)
    step_off_b = np.zeros(WPC, np.int64)
    blk_base = 0
    i0 = 0
    for j in range(N_BLKS):
        steps = BLOCK_PLAN[j]
        na_blk = int(na_step[i0 : i0 + steps].sum())
        o = blk_base
        for i in range(i0, i0 + steps):
            step_off_a[i] = o
            o += na_step[i]
        o = blk_base + na_blk
        for i in range(i0, i0 + steps):
            step_off_b[i] = o
            o += nb_step[i]
        blk_base = o
        i0 += steps
    T_tot = int(blk_base)

    ea16 = np.asarray(edata_a).astype(E3M4)
    eb16 = np.asarray(edata_b).astype(E3M4)

    vdata = np.asarray(vdata)
    vpad = np.zeros((NTOT, D_NODE), dtype=np.float32)
    vpad[:N_NODES] = vdata

    blk_tot = []
    i0 = 0
    for j in range(N_BLKS):
        steps = BLOCK_PLAN[j]
        blk_tot.append(
            int(na_step[i0 : i0 + steps].sum() + nb_step[i0 : i0 + steps].sum())
        )
        i0 += steps
    max_blk = max(blk_tot)

    iota64 = np.ascontiguousarray(
        np.broadcast_to(np.arange(WIN, dtype=np.float32), (128, WIN))
    ).astype(BF16)
    Wf = np.ascontiguousarray(np.asarray(W_mat), dtype=np.float32).astype(BF16)
    bf = np.asarray(b_vec).astype(np.float32).reshape(D_OUT, 1)

    in_maps = []
    for c in range(N_CORES):
        slot_eid = np.full(T_tot * 128, -1, dtype=np.int64)
        slot_rel = np.full(T_tot * 128, -1.0, dtype=np.float32)
        slot_is_a = np.zeros(T_tot * 128, dtype=bool)
        for i in range(WPC):
            g = gwins[c][i]
            for ids, starts, cnts, soff, recv, is_a in (
                (ids_a, st_a, cnt_a, step_off_a, recv_a, True),
                (ids_b, st_b, cnt_b, step_off_b, recv_b, False),
            ):
                cnt = cnts[g]
                if cnt == 0:
                    continue
                eids = ids[starts[g] : starts[g] + cnt]
                s0 = soff[i] * 128
                slot_eid[s0 : s0 + cnt] = eids
                slot_is_a[s0 : s0 + cnt] = is_a
                slot_rel[s0 : s0 + cnt] = (recv[eids] & (WIN - 1)).astype(
                    np.float32
                )
        idx = np.maximum(slot_eid, 0)
        gath = np.where(slot_is_a[:, None], ea16[idx], eb16[idx])
        gath[slot_eid < 0] = 0
        eh = np.ascontiguousarray(
            gath.reshape(T_tot, 128, 64).transpose(1, 0, 2)
        )  # [slot, tile, feat] bf16
        relT = slot_rel.reshape(T_tot, 128).T.astype(BF16)
        rel2 = np.ascontiguousarray(np.repeat(relT, 2, axis=1))  # [128, 2T]
        nodes = (
            gwins[c][:, None] * WIN + np.arange(WIN)[None, :]
        ).reshape(-1)
        vT = np.ascontiguousarray(
            vpad[nodes].T.astype(E3M4)
        )  # [128, NPC] fp8 e3m4
        in_maps.append(
            {"eh": eh, "rel": rel2, "vT": vT, "Wd": Wf, "bd": bf,
             "iota": iota64}
        )

    sched = (tuple(int(x) for x in na_step), tuple(int(x) for x in nb_step))
    return in_maps, sched, gwins


# ---------------------------------------------------------------------------
# device kernel
# ---------------------------------------------------------------------------

_NC_CACHE = {}


def _build(sched):
    na_step, nb_step = sched
    f32 = mybir.dt.float32
    bf16 = mybir.dt.bfloat16

    # packed per-block layout: [a tiles | b tiles] per block
    blk_na = []
    blk_nb = []
    blk_i0 = []
    i0 = 0
    for j in range(N_BLKS):
        steps = BLOCK_PLAN[j]
        blk_i0.append(i0)
        blk_na.append(sum(na_step[i0 : i0 + steps]))
        blk_nb.append(sum(nb_step[i0 : i0 + steps]))
        i0 += steps
    blk_tot = [a + b for a, b in zip(blk_na, blk_nb)]
    max_blk = max(blk_tot)
    T_tot = sum(blk_tot)

    nc = bass.Bass(trn_type="TRN2")
    fp8 = mybir.dt.float8e3
    eh_d = nc.dram_tensor("eh", [128, T_tot, 64], fp8, kind="ExternalInput")
    rel_d = nc.dram_tensor("rel", [128, 2 * T_tot], bf16, kind="ExternalInput")
    vT_d = nc.dram_tensor("vT", [128, NPC], fp8, kind="ExternalInput")
    W_d = nc.dram_tensor("Wd", [2 * D_NODE, D_OUT], bf16, kind="ExternalInput")
    b_d = nc.dram_tensor("bd", [D_OUT, 1], f32, kind="ExternalInput")
    iota_d = nc.dram_tensor("iota", [128, WIN], bf16, kind="ExternalInput")
    outT_d = nc.dram_tensor("outT", [128, NPC], bf16, kind="ExternalOutput")

    with tile.TileContext(nc) as tc:
        with (
            tc.tile_pool(name="consts", bufs=1) as cb,
            tc.tile_pool(name="x0", bufs=3) as x0p,
            tc.tile_pool(name="edges", bufs=5) as ep,
            tc.tile_pool(name="sel", bufs=4) as sp,
            tc.tile_pool(name="out", bufs=2) as op,
            tc.tile_pool(name="psum1", bufs=4, space="PSUM") as pp1,
            tc.tile_pool(name="psum2", bufs=2, space="PSUM") as pp2,
        ):
            iota_sb = cb.tile([128, WIN], bf16)
            nc.scalar.dma_start(iota_sb[:], iota_d[:, :])
            w0_sb = cb.tile([128, D_OUT], bf16, tag="w0")
            nc.scalar.dma_start(w0_sb[:], W_d[0:128, :])
            w1_sb = cb.tile([128, D_OUT], bf16, tag="w1")
            nc.scalar.dma_start(w1_sb[:], W_d[128:256, :])
            b_sb = cb.tile([D_OUT, 1], f32, tag="b")
            nc.scalar.dma_start(b_sb[:], b_d[:, :])
            rel_sb = cb.tile([128, 2 * T_tot], bf16, tag="rel")
            rel_head = 2 * sum(blk_tot[:6])
            nc.scalar.dma_start(rel_sb[:, :rel_head], rel_d[:, :rel_head])
            vt_sb = cb.tile([128, NPC], fp8, tag="vt")

            off = 0
            ot = None
            chunk_col0 = 0
            ot_cols = 0
            chunk_starts = set()
            s = 0
            for n in OUT_PLAN:
                chunk_starts.add(s)
                s += n
            for j in range(N_BLKS):
                i0 = blk_i0[j]
                steps = BLOCK_PLAN[j]
                cols_blk = steps * WIN
                n_blk = blk_tot[j]
                na_b = blk_na[j]

                # one coalesced edge DMA per block (~1.2 MB)
                et = ep.tile([128, max_blk * 64], fp8, tag="et")
                nc.sync.dma_start(
                    et[:, : n_blk * 64], eh_d[:, off : off + n_blk, :]
                )
                if j == 1:
                    nc.scalar.dma_start(
                        rel_sb[:, rel_head:], rel_d[:, rel_head:]
                    )
                # vT arrives in 5 chunks spread over the early blocks
                if j < 10 and j % 2 == 0:
                    k = j // 2
                    vc0 = k * (NPC // 5)
                    vc1 = NPC if k == 4 else (k + 1) * (NPC // 5)
                    nc.scalar.dma_start(vt_sb[:, vc0:vc1], vT_d[:, vc0:vc1])

                selb = sp.tile([128, max_blk * WIN], bf16, tag="selb")
                # rel2 dup-x2 keeps a dense 64-bit innermost run -> 2x uop
                in0 = rel_sb[:, 2 * off : 2 * (off + n_blk)].rearrange(
                    "p (n one d) -> p n one d", one=1, d=2
                ).broadcast_to([128, n_blk, WIN // 2, 2])
                in1 = iota_sb[:].rearrange(
                    "p (w d) -> p w d", d=2
                ).rearrange(
                    "p (one w) d -> p one w d", one=1
                ).broadcast_to([128, n_blk, WIN // 2, 2])
                nc.vector.tensor_tensor(
                    out=selb[:, : n_blk * WIN].rearrange(
                        "p (n w d) -> p n w d", w=WIN // 2, d=2
                    ),
                    in0=in0, in1=in1, op=mybir.AluOpType.is_equal,
                )

                x0 = x0p.tile([128, BLK_STEPS * WIN], bf16, tag="x0")
                ps = pp1.tile([128, BLK_STEPS * WIN], f32, tag="p1")
                t = 0
                for half, n_stp in ((0, na_step), (1, nb_step)):
                    r0 = half * 64
                    tt = 0
                    n_half = blk_na[j] if half == 0 else blk_nb[j]
                    for stp in range(steps):
                        for k in range(n_stp[i0 + stp]):
                            nc.tensor.matmul(
                                out=ps[
                                    r0 : r0 + 64,
                                    stp * WIN : (stp + 1) * WIN,
                                ],
                                lhsT=et[:, t * 64 : (t + 1) * 64],
                                rhs=selb[:, t * WIN : (t + 1) * WIN],
                                start=(tt == 0),
                                stop=(tt == n_half - 1),
                            )
                            t += 1
                            tt += 1
                nc.scalar.copy(x0[:, :cols_blk], ps[:, :cols_blk])
                off += n_blk

                po = pp2.tile([128, BLK_STEPS * WIN], f32, tag="p2")
                nc.tensor.matmul(
                    out=po[:, :cols_blk], lhsT=w0_sb[:], rhs=x0[:, :cols_blk],
                    start=True, stop=False,
                )
                nc.tensor.matmul(
                    out=po[:, :cols_blk],
                    lhsT=w1_sb[:],
                    rhs=vt_sb[:, i0 * WIN : i0 * WIN + cols_blk],
                    start=False, stop=True,
                )
                # outT staged per OUT_PLAN chunk, then one store each
                if j in chunk_starts:
                    ot = op.tile(
                        [128, max(OUT_PLAN) * BLK_STEPS * WIN], bf16, tag="ot"
                    )
                    chunk_col0 = i0 * WIN
                    ot_cols = 0
                nc.scalar.activation(
                    out=ot[:, ot_cols : ot_cols + cols_blk],
                    in_=po[:, :cols_blk],
                    func=mybir.ActivationFunctionType.Identity,
                    bias=b_sb[:, 0:1],
                    scale=1.0,
                )
                ot_cols += cols_blk
                if j + 1 in chunk_starts or j == N_BLKS - 1:
                    nc.scalar.dma_start(
                        outT_d[:, chunk_col0 : chunk_col0 + ot_cols],
                        ot[:, :ot_cols],
                    )
    _split_multi_waits(nc)
    return nc


# ---------------------------------------------------------------------------
# public entry point
# ---------------------------------------------------------------------------


def kernel(vdata, edata_a, edata_b, conn_a, conn_b, W, b, _trace=False):
    in_maps, sched, gwins = _preprocess(
        vdata, edata_a, edata_b, conn_a, conn_b, W, b
    )
    nc = _NC_CACHE.get(sched)
    if nc is None:
        nc = _build(sched)
        _NC_CACHE[sched] = nc
    kwargs = {}
    if _trace:
        kwargs = dict(trace=True, trace_cores=[0])
    res = run_bass_kernel_spmd(
        nc, in_maps, core_ids=list(range(N_CORES)), **kwargs
    )

    out_full = np.empty((NTOT, D_OUT), dtype=np.float32)
    for c in range(N_CORES):
        outT = res.results[c]["outT"].astype(np.float32)  # [128, NPC] bf16
        blocks = outT.reshape(D_OUT, WPC, WIN).transpose(1, 2, 0)  # [i, w, d]
        # scatter step blocks back to natural (global window) order
        dst = gwins[c][:, None] * WIN + np.arange(WIN)[None, :]
        out_full[dst.reshape(-1)] = blocks.reshape(NPC, D_OUT)
    out = out_full[:N_NODES]
    if _trace:
        return out, res
    return out

